# revision 3
# baseline (speedup 1.0000x reference)
"""nn_Dense_Local fixed-point dense layer on 8 TRN2 NeuronCores.

y = fxp(relu(fxp(fxp(x) @ fxp(w)) + fxp(b))), fxp = round-to-nearest-even on the
2^-16 grid.

Sharding: tensor-parallel over output columns (n). Each of the 8 cores gets the
full x (pre-transposed on host to xT for the stationary operand) and a
[4096, 512] column shard of w; core i computes y[:, 512*i : 512*(i+1)].

Math mode "f32r_2p": exact-by-construction Dekker split of x into two fp32r
(12-significand-bit) chunks against fp32r-quantized w; 2 matmul passes, each at
full PE rate. Quantized W ints fit 12 bits except ~6e-5 of entries (|W|>4096),
whose half-ulp rounding contributes ~5e-6 relative L2 — far below any
plausible tolerance.

Math mode "f16_3p": fp16 Dekker 3-pass (xh.wh + xl.wh + xh.wl), absmax vs
fp64 reference < 0.2 grid steps; ~50% more PE time.
"""

import numpy as np

import concourse.bass as bass
import concourse.bacc as bacc
import concourse.mybir as mybir
import concourse.tile as tile
from concourse.bass_utils import run_bass_kernel_spmd

P = 128
BATCH = 2048
IN_DIM = 4096
OUT_DIM = 4096
N_CORES = 8

N_SHARD = OUT_DIM // N_CORES       # 512 columns per core
KT = IN_DIM // P                   # 32 k-tiles
MT = BATCH // P                    # 16 m-tiles
C_MAGIC = 3 * (2.0 ** 22)          # fp32 round-to-int magic constant
INV_S = 1.0 / 65536.0

MODE = "f32r_2p"                   # "f32r_2p" | "f16_3p"

_CACHE = {}


def _build(mode):
    nc = bacc.Bacc(trn_type="TRN2", target_bir_lowering=False)
    xt = nc.dram_tensor("xt", [IN_DIM, BATCH], mybir.dt.float32, kind="ExternalInput")
    w = nc.dram_tensor("w", [IN_DIM, N_SHARD], mybir.dt.float32, kind="ExternalInput")
    b = nc.dram_tensor("b", [N_SHARD], mybir.dt.float32, kind="ExternalInput")
    y = nc.dram_tensor("y", [BATCH, N_SHARD], mybir.dt.float32, kind="ExternalOutput")

    w_r = w.rearrange("(kt p) n -> kt p n", p=P)      # [32, 128, 512]

    KC = 4                                            # k-tiles per w-prep chunk
    f32r = mybir.dt.float32r
    f16 = mybir.dt.float16
    f32 = mybir.dt.float32
    ACT_COPY = mybir.ActivationFunctionType.Copy

    with tile.TileContext(nc) as tc:
        with (
            tc.tile_pool(name="wres", bufs=1) as wres,
            tc.tile_pool(name="wtmp", bufs=2) as wtmp,
            tc.tile_pool(name="xload", bufs=2) as xload,
            tc.tile_pool(name="xchunk", bufs=2) as xchunk,
            tc.tile_pool(name="epi", bufs=2) as epi,
            tc.tile_pool(name="const", bufs=1) as cpool,
            tc.tile_pool(name="psum", bufs=4, space="PSUM") as psum,
        ):
            # ---- bias: broadcast to all partitions, quantize to grid
            b_sb = cpool.tile([P, N_SHARD], f32, tag="b_sb")
            b_ap = b[:]
            b_bcast = bass.AP(
                tensor=b_ap.tensor, offset=b_ap.offset,
                ap=[[0, P]] + [list(s) for s in b_ap.ap],
            )
            nc.gpsimd.dma_start(out=b_sb[:], in_=b_bcast)
            nc.scalar.activation(b_sb[:], b_sb[:], ACT_COPY, bias=C_MAGIC, scale=65536.0)
            nc.scalar.activation(b_sb[:], b_sb[:], ACT_COPY, bias=-C_MAGIC, scale=1.0)
            nc.scalar.activation(b_sb[:], b_sb[:], ACT_COPY, bias=0.0, scale=INV_S)

            # ---- w prep: quantize to W ints, emit per-mode operand tiles
            if mode == "f32r_2p":
                wr = wres.tile([P, KT, N_SHARD], f32r, tag="wr")
            else:
                wh = wres.tile([P, KT, N_SHARD], f16, tag="wh")
                wl = wres.tile([P, KT, N_SHARD], f16, tag="wl")

            for c in range(KT // KC):
                wt = wtmp.tile([P, KC, N_SHARD], f32, tag="wt")
                for j in range(KC):
                    nc.sync.dma_start(wt[:, j, :], w_r[c * KC + j])
                # wt := round(w * 2^16)  (integer-valued W)
                nc.vector.tensor_scalar(wt[:], wt[:], 65536.0, C_MAGIC,
                                        mybir.AluOpType.mult, mybir.AluOpType.add)
                nc.scalar.activation(wt[:], wt[:], ACT_COPY, bias=-C_MAGIC, scale=1.0)
                ks = slice(c * KC, (c + 1) * KC)
                if mode == "f32r_2p":
                    nc.scalar.activation(wr[:, ks, :], wt[:], ACT_COPY,
                                         bias=0.0, scale=INV_S)
                else:
                    nc.scalar.activation(wh[:, ks, :], wt[:], ACT_COPY,
                                         bias=0.0, scale=INV_S)
                    nc.vector.scalar_tensor_tensor(wl[:, ks, :], wt[:], INV_S,
                                                   wh[:, ks, :],
                                                   mybir.AluOpType.mult,
                                                   mybir.AluOpType.subtract)

            # ---- main loop over m-tiles
            for m in range(MT):
                # load + split x m-tile (all 32 k-tiles at once, FD=4096)
                # xt[(kt p), m*128 : (m+1)*128] -> [p, kt, 128]
                xt_sl = xt[:, m * P:(m + 1) * P].rearrange("(kt p) m -> p kt m", p=P)
                xd = f32r if mode == "f32r_2p" else f16
                xa = xchunk.tile([P, KT, P], xd, tag="xa")
                xb = xchunk.tile([P, KT, P], xd, tag="xb")
                XC = 8                      # k-tiles per x load/split chunk
                for h in range(KT // XC):
                    ks = slice(h * XC, (h + 1) * XC)
                    xf = xload.tile([P, XC, P], f32, tag="xf")
                    nc.sync.dma_start(xf[:], xt_sl[:, ks, :])
                    nc.scalar.copy(xa[:, ks, :], xf[:])
                    nc.vector.tensor_sub(xb[:, ks, :], xf[:], xa[:, ks, :])
                if mode == "f32r_2p":
                    mms = [(xa, wr), (xb, wr)]
                else:
                    mms = [(xa, wh), (xb, wh), (xa, wl)]

                pt = psum.tile([P, N_SHARD], f32, tag="pt")
                n_mm = len(mms) * KT
                i = 0
                for k in range(KT):
                    for (lh, rh) in mms:
                        nc.tensor.matmul(pt[:], lh[:, k, :], rh[:, k, :],
                                         start=(i == 0), stop=(i == n_mm - 1))
                        i += 1

                # epilogue: fxp(mm) + bq, relu (already on grid -> final fxp is identity)
                t = epi.tile([P, N_SHARD], f32, tag="t")
                nc.scalar.activation(t[:], pt[:], ACT_COPY, bias=C_MAGIC, scale=65536.0)
                nc.scalar.activation(t[:], t[:], ACT_COPY, bias=-C_MAGIC, scale=1.0)
                y2 = epi.tile([P, N_SHARD], f32, tag="y2")
                nc.vector.scalar_tensor_tensor(y2[:], t[:], INV_S, b_sb[:],
                                               mybir.AluOpType.mult,
                                               mybir.AluOpType.add)
                nc.vector.tensor_single_scalar(y2[:], y2[:], 0.0, mybir.AluOpType.max)
                nc.sync.dma_start(y[m * P:(m + 1) * P, :], y2[:])
    nc.finalize()
    return nc


def kernel(x, w, b):
    x = np.ascontiguousarray(x, dtype=np.float32)
    w = np.ascontiguousarray(w, dtype=np.float32)
    b = np.ascontiguousarray(b, dtype=np.float32)
    assert x.shape == (BATCH, IN_DIM) and w.shape == (IN_DIM, OUT_DIM)

    if MODE not in _CACHE:
        _CACHE[MODE] = _build(MODE)
    nc = _CACHE[MODE]

    xt = np.ascontiguousarray(x.T)
    in_maps = []
    for i in range(N_CORES):
        sl = slice(i * N_SHARD, (i + 1) * N_SHARD)
        in_maps.append({
            "xt": xt,
            "w": np.ascontiguousarray(w[:, sl]),
            "b": np.ascontiguousarray(b[sl]),
        })
    res = run_bass_kernel_spmd(nc, in_maps, core_ids=list(range(N_CORES)))
    out = np.empty((BATCH, OUT_DIM), dtype=np.float32)
    for i in range(N_CORES):
        out[:, i * N_SHARD:(i + 1) * N_SHARD] = res.results[i]["y"]
    return out


# revision 4
# speedup vs baseline: 37403.3542x; 37403.3542x over previous
"""nn_Dense_Local fixed-point dense layer on 8 TRN2 NeuronCores.

y = fxp(relu(fxp(fxp(x) @ fxp(w)) + fxp(b))), fxp = round-to-nearest-even on the
2^-16 grid.

Sharding: tensor-parallel over output columns (n). Each of the 8 cores gets the
full x (pre-transposed on host to xT for the stationary operand) and a
[4096, 512] column shard of w; core i computes y[:, 512*i : 512*(i+1)].

Math mode "f32r_2p": exact-by-construction Dekker split of x into two fp32r
(12-significand-bit) chunks against fp32r-quantized w; 2 matmul passes, each at
full PE rate. Quantized W ints fit 12 bits except ~6e-5 of entries (|W|>4096),
whose half-ulp rounding contributes ~5e-6 relative L2 — far below any
plausible tolerance.

Math mode "f16_3p": fp16 Dekker 3-pass (xh.wh + xl.wh + xh.wl), absmax vs
fp64 reference < 0.2 grid steps; ~50% more PE time.
"""

import numpy as np

import concourse.bass as bass
import concourse.bacc as bacc
import concourse.mybir as mybir
import concourse.tile as tile
from concourse.bass_utils import run_bass_kernel_spmd

P = 128
BATCH = 2048
IN_DIM = 4096
OUT_DIM = 4096
N_CORES = 8

N_SHARD = OUT_DIM // N_CORES       # 512 columns per core
KT = IN_DIM // P                   # 32 k-tiles
MT = BATCH // P                    # 16 m-tiles
C_MAGIC = 3 * (2.0 ** 22)          # fp32 round-to-int magic constant
INV_S = 1.0 / 65536.0

MODE = "f32r_2p"                   # "f32r_2p" | "f16_3p"

_CACHE = {}


def _build(mode, reps=1):
    nc = bacc.Bacc(trn_type="TRN2", target_bir_lowering=False)
    xt = nc.dram_tensor("xt", [IN_DIM, BATCH], mybir.dt.float32, kind="ExternalInput")
    w = nc.dram_tensor("w", [IN_DIM, N_SHARD], mybir.dt.float32, kind="ExternalInput")
    b = nc.dram_tensor("b", [N_SHARD], mybir.dt.float32, kind="ExternalInput")
    y = nc.dram_tensor("y", [BATCH, N_SHARD], mybir.dt.float32, kind="ExternalOutput")

    w_r = w.rearrange("(kt p) n -> kt p n", p=P)      # [32, 128, 512]

    KC = 4                                            # k-tiles per w-prep chunk
    f32r = mybir.dt.float32r
    f16 = mybir.dt.float16
    f32 = mybir.dt.float32
    ACT_COPY = mybir.ActivationFunctionType.Copy

    import contextlib

    with tile.TileContext(nc) as tc:
        loop_cm = tc.For_i(0, reps, 1) if reps > 1 else contextlib.nullcontext()
        with (
            tc.tile_pool(name="wres", bufs=1) as wres,
            tc.tile_pool(name="wtmp", bufs=2) as wtmp,
            tc.tile_pool(name="xload", bufs=2) as xload,
            tc.tile_pool(name="xchunk", bufs=2) as xchunk,
            tc.tile_pool(name="epi", bufs=2) as epi,
            tc.tile_pool(name="const", bufs=1) as cpool,
            tc.tile_pool(name="psum", bufs=4, space="PSUM") as psum,
            loop_cm,
        ):
            # ---- bias: broadcast to all partitions, quantize to grid
            b_sb = cpool.tile([P, N_SHARD], f32, tag="b_sb")
            b_ap = b[:]
            b_bcast = bass.AP(
                tensor=b_ap.tensor, offset=b_ap.offset,
                ap=[[0, P]] + [list(s) for s in b_ap.ap],
            )
            nc.gpsimd.dma_start(out=b_sb[:], in_=b_bcast)
            nc.scalar.activation(b_sb[:], b_sb[:], ACT_COPY, bias=C_MAGIC, scale=65536.0)
            nc.scalar.activation(b_sb[:], b_sb[:], ACT_COPY, bias=-C_MAGIC, scale=1.0)
            nc.scalar.activation(b_sb[:], b_sb[:], ACT_COPY, bias=0.0, scale=INV_S)

            # ---- w prep: quantize to W ints, emit per-mode operand tiles
            if mode == "f32r_2p":
                wr = wres.tile([P, KT, N_SHARD], f32r, tag="wr")
            else:
                wh = wres.tile([P, KT, N_SHARD], f16, tag="wh")
                wl = wres.tile([P, KT, N_SHARD], f16, tag="wl")

            for c in range(KT // KC):
                wt = wtmp.tile([P, KC, N_SHARD], f32, tag="wt")
                for j in range(KC):
                    nc.sync.dma_start(wt[:, j, :], w_r[c * KC + j])
                # wt := round(w * 2^16)  (integer-valued W)
                nc.vector.tensor_scalar(wt[:], wt[:], 65536.0, C_MAGIC,
                                        mybir.AluOpType.mult, mybir.AluOpType.add)
                nc.scalar.activation(wt[:], wt[:], ACT_COPY, bias=-C_MAGIC, scale=1.0)
                ks = slice(c * KC, (c + 1) * KC)
                if mode == "f32r_2p":
                    nc.scalar.activation(wr[:, ks, :], wt[:], ACT_COPY,
                                         bias=0.0, scale=INV_S)
                else:
                    nc.scalar.activation(wh[:, ks, :], wt[:], ACT_COPY,
                                         bias=0.0, scale=INV_S)
                    nc.vector.scalar_tensor_tensor(wl[:, ks, :], wt[:], INV_S,
                                                   wh[:, ks, :],
                                                   mybir.AluOpType.mult,
                                                   mybir.AluOpType.subtract)

            # ---- main loop over m-tiles
            for m in range(MT):
                # load + split x m-tile (all 32 k-tiles at once, FD=4096)
                # xt[(kt p), m*128 : (m+1)*128] -> [p, kt, 128]
                xt_sl = xt[:, m * P:(m + 1) * P].rearrange("(kt p) m -> p kt m", p=P)
                xd = f32r if mode == "f32r_2p" else f16
                xa = xchunk.tile([P, KT, P], xd, tag="xa")
                xb = xchunk.tile([P, KT, P], xd, tag="xb")
                XC = 8                      # k-tiles per x load/split chunk
                for h in range(KT // XC):
                    ks = slice(h * XC, (h + 1) * XC)
                    xf = xload.tile([P, XC, P], f32, tag="xf")
                    nc.sync.dma_start(xf[:], xt_sl[:, ks, :])
                    nc.scalar.copy(xa[:, ks, :], xf[:])
                    nc.vector.tensor_sub(xb[:, ks, :], xf[:], xa[:, ks, :])
                if mode == "f32r_2p":
                    mms = [(xa, wr), (xb, wr)]
                else:
                    mms = [(xa, wh), (xb, wh), (xa, wl)]

                pt = psum.tile([P, N_SHARD], f32, tag="pt")
                n_mm = len(mms) * KT
                i = 0
                for k in range(KT):
                    for (lh, rh) in mms:
                        nc.tensor.matmul(pt[:], lh[:, k, :], rh[:, k, :],
                                         start=(i == 0), stop=(i == n_mm - 1))
                        i += 1

                # epilogue: fxp(mm) + bq, relu (already on grid -> final fxp is identity)
                t = epi.tile([P, N_SHARD], f32, tag="t")
                nc.scalar.activation(t[:], pt[:], ACT_COPY, bias=C_MAGIC, scale=65536.0)
                nc.scalar.activation(t[:], t[:], ACT_COPY, bias=-C_MAGIC, scale=1.0)
                y2 = epi.tile([P, N_SHARD], f32, tag="y2")
                nc.vector.scalar_tensor_tensor(y2[:], t[:], INV_S, b_sb[:],
                                               mybir.AluOpType.mult,
                                               mybir.AluOpType.add)
                nc.vector.tensor_single_scalar(y2[:], y2[:], 0.0, mybir.AluOpType.max)
                nc.sync.dma_start(y[m * P:(m + 1) * P, :], y2[:])
    nc.finalize()
    return nc


def kernel(x, w, b):
    x = np.ascontiguousarray(x, dtype=np.float32)
    w = np.ascontiguousarray(w, dtype=np.float32)
    b = np.ascontiguousarray(b, dtype=np.float32)
    assert x.shape == (BATCH, IN_DIM) and w.shape == (IN_DIM, OUT_DIM)

    if MODE not in _CACHE:
        _CACHE[MODE] = _build(MODE)
    nc = _CACHE[MODE]

    xt = np.ascontiguousarray(x.T)
    in_maps = []
    for i in range(N_CORES):
        sl = slice(i * N_SHARD, (i + 1) * N_SHARD)
        in_maps.append({
            "xt": xt,
            "w": np.ascontiguousarray(w[:, sl]),
            "b": np.ascontiguousarray(b[sl]),
        })
    res = run_bass_kernel_spmd(nc, in_maps, core_ids=list(range(N_CORES)))
    out = np.empty((BATCH, OUT_DIM), dtype=np.float32)
    for i in range(N_CORES):
        out[:, i * N_SHARD:(i + 1) * N_SHARD] = res.results[i]["y"]
    return out


# revision 5
# speedup vs baseline: 40102.9024x; 1.0722x over previous
"""nn_Dense_Local fixed-point dense layer on 8 TRN2 NeuronCores.

y = fxp(relu(fxp(fxp(x) @ fxp(w)) + fxp(b))), fxp = round-to-nearest-even on the
2^-16 grid.

Sharding: tensor-parallel over output columns (n). Each of the 8 cores gets the
full x (pre-transposed on host to xT for the stationary operand) and a
[4096, 512] column shard of w; core i computes y[:, 512*i : 512*(i+1)].

Math mode "f32r_2p": exact-by-construction Dekker split of x into two fp32r
(12-significand-bit) chunks against fp32r-quantized w; 2 matmul passes, each at
full PE rate. Quantized W ints fit 12 bits except ~6e-5 of entries (|W|>4096),
whose half-ulp rounding contributes ~5e-6 relative L2 — far below any
plausible tolerance.

Math mode "f16_3p": fp16 Dekker 3-pass (xh.wh + xl.wh + xh.wl), absmax vs
fp64 reference < 0.2 grid steps; ~50% more PE time.
"""

import numpy as np

import concourse.bass as bass
import concourse.bacc as bacc
import concourse.mybir as mybir
import concourse.tile as tile
from concourse.bass_utils import run_bass_kernel_spmd

P = 128
BATCH = 2048
IN_DIM = 4096
OUT_DIM = 4096
N_CORES = 8

N_SHARD = OUT_DIM // N_CORES       # 512 columns per core
KT = IN_DIM // P                   # 32 k-tiles
MT = BATCH // P                    # 16 m-tiles
C_MAGIC = 3 * (2.0 ** 22)          # fp32 round-to-int magic constant
INV_S = 1.0 / 65536.0

MODE = "f32r_2p"                   # "f32r_2p" | "f16_3p"

_CACHE = {}


def _build(mode, reps=1):
    nc = bacc.Bacc(trn_type="TRN2", target_bir_lowering=False)
    xt = nc.dram_tensor("xt", [IN_DIM, BATCH], mybir.dt.float32, kind="ExternalInput")
    w = nc.dram_tensor("w", [IN_DIM, N_SHARD], mybir.dt.float32, kind="ExternalInput")
    b = nc.dram_tensor("b", [N_SHARD], mybir.dt.float32, kind="ExternalInput")
    y = nc.dram_tensor("y", [BATCH, N_SHARD], mybir.dt.float32, kind="ExternalOutput")

    w_r = w.rearrange("(kt p) n -> kt p n", p=P)      # [32, 128, 512]

    KC = 4                                            # k-tiles per w-prep chunk
    f32r = mybir.dt.float32r
    f16 = mybir.dt.float16
    f32 = mybir.dt.float32
    ACT_COPY = mybir.ActivationFunctionType.Copy

    import contextlib

    with tile.TileContext(nc) as tc:
        loop_cm = tc.For_i(0, reps, 1) if reps > 1 else contextlib.nullcontext()
        with (
            tc.tile_pool(name="wres", bufs=1) as wres,
            tc.tile_pool(name="wtmp", bufs=2) as wtmp,
            tc.tile_pool(name="xload", bufs=2) as xload,
            tc.tile_pool(name="xchunk", bufs=2) as xchunk,
            tc.tile_pool(name="epi", bufs=2) as epi,
            tc.tile_pool(name="const", bufs=1) as cpool,
            tc.tile_pool(name="psum", bufs=4, space="PSUM") as psum,
            loop_cm,
        ):
            # ---- bias: broadcast to all partitions, quantize to grid
            b_sb = cpool.tile([P, N_SHARD], f32, tag="b_sb")
            b_ap = b[:]
            b_bcast = bass.AP(
                tensor=b_ap.tensor, offset=b_ap.offset,
                ap=[[0, P]] + [list(s) for s in b_ap.ap],
            )
            nc.gpsimd.dma_start(out=b_sb[:], in_=b_bcast)
            nc.scalar.activation(b_sb[:], b_sb[:], ACT_COPY, bias=C_MAGIC, scale=65536.0)
            # (t - C) * 2^-16 == t * 2^-16 - 192 exactly (t*2^-16 is a multiple of
            # 2^-16 near 192, where the fp32 ulp is 2^-16)
            nc.scalar.activation(b_sb[:], b_sb[:], ACT_COPY, bias=-192.0, scale=INV_S)

            # ---- w prep: quantize to W ints, emit per-mode operand tiles
            if mode == "f32r_2p":
                wr = wres.tile([P, KT, N_SHARD], f32r, tag="wr")
            else:
                wh = wres.tile([P, KT, N_SHARD], f16, tag="wh")
                wl = wres.tile([P, KT, N_SHARD], f16, tag="wl")

            for c in range(KT // KC):
                wt = wtmp.tile([P, KC, N_SHARD], f32, tag="wt")
                for j in range(KC):
                    nc.sync.dma_start(wt[:, j, :], w_r[c * KC + j])
                # wt := round(w * 2^16)  (integer-valued W)
                nc.vector.tensor_scalar(wt[:], wt[:], 65536.0, C_MAGIC,
                                        mybir.AluOpType.mult, mybir.AluOpType.add)
                ks = slice(c * KC, (c + 1) * KC)
                if mode == "f32r_2p":
                    nc.scalar.activation(wr[:, ks, :], wt[:], ACT_COPY,
                                         bias=-192.0, scale=INV_S)
                else:
                    nc.scalar.activation(wt[:], wt[:], ACT_COPY, bias=-C_MAGIC,
                                         scale=1.0)
                    nc.scalar.activation(wh[:, ks, :], wt[:], ACT_COPY,
                                         bias=0.0, scale=INV_S)
                    nc.vector.scalar_tensor_tensor(wl[:, ks, :], wt[:], INV_S,
                                                   wh[:, ks, :],
                                                   mybir.AluOpType.mult,
                                                   mybir.AluOpType.subtract)

            # ---- main loop over m-tiles
            for m in range(MT):
                # load + split x m-tile (all 32 k-tiles at once, FD=4096)
                # xt[(kt p), m*128 : (m+1)*128] -> [p, kt, 128]
                xt_sl = xt[:, m * P:(m + 1) * P].rearrange("(kt p) m -> p kt m", p=P)
                xd = f32r if mode == "f32r_2p" else f16
                xa = xchunk.tile([P, KT, P], xd, tag="xa")
                xb = xchunk.tile([P, KT, P], xd, tag="xb")
                XC = 8                      # k-tiles per x load/split chunk
                for h in range(KT // XC):
                    ks = slice(h * XC, (h + 1) * XC)
                    xf = xload.tile([P, XC, P], f32, tag="xf")
                    nc.sync.dma_start(xf[:], xt_sl[:, ks, :])
                    nc.vector.tensor_copy(xa[:, ks, :], xf[:])
                    nc.vector.tensor_sub(xb[:, ks, :], xf[:], xa[:, ks, :])
                if mode == "f32r_2p":
                    mms = [(xa, wr), (xb, wr)]
                else:
                    mms = [(xa, wh), (xb, wh), (xa, wl)]

                pt = psum.tile([P, N_SHARD], f32, tag="pt")
                n_mm = len(mms) * KT
                i = 0
                for k in range(KT):
                    for (lh, rh) in mms:
                        nc.tensor.matmul(pt[:], lh[:, k, :], rh[:, k, :],
                                         start=(i == 0), stop=(i == n_mm - 1))
                        i += 1

                # epilogue: fxp(mm) + bq, relu (already on grid -> final fxp is identity)
                t = epi.tile([P, N_SHARD], f32, tag="t")
                nc.scalar.activation(t[:], pt[:], ACT_COPY, bias=C_MAGIC, scale=65536.0)
                nc.scalar.activation(t[:], t[:], ACT_COPY, bias=-C_MAGIC, scale=1.0)
                y2 = epi.tile([P, N_SHARD], f32, tag="y2")
                nc.vector.scalar_tensor_tensor(y2[:], t[:], INV_S, b_sb[:],
                                               mybir.AluOpType.mult,
                                               mybir.AluOpType.add)
                nc.vector.tensor_single_scalar(y2[:], y2[:], 0.0, mybir.AluOpType.max)
                nc.sync.dma_start(y[m * P:(m + 1) * P, :], y2[:])
    nc.finalize()
    return nc


def kernel(x, w, b):
    x = np.ascontiguousarray(x, dtype=np.float32)
    w = np.ascontiguousarray(w, dtype=np.float32)
    b = np.ascontiguousarray(b, dtype=np.float32)
    assert x.shape == (BATCH, IN_DIM) and w.shape == (IN_DIM, OUT_DIM)

    if MODE not in _CACHE:
        _CACHE[MODE] = _build(MODE)
    nc = _CACHE[MODE]

    xt = np.ascontiguousarray(x.T)
    in_maps = []
    for i in range(N_CORES):
        sl = slice(i * N_SHARD, (i + 1) * N_SHARD)
        in_maps.append({
            "xt": xt,
            "w": np.ascontiguousarray(w[:, sl]),
            "b": np.ascontiguousarray(b[sl]),
        })
    res = run_bass_kernel_spmd(nc, in_maps, core_ids=list(range(N_CORES)))
    out = np.empty((BATCH, OUT_DIM), dtype=np.float32)
    for i in range(N_CORES):
        out[:, i * N_SHARD:(i + 1) * N_SHARD] = res.results[i]["y"]
    return out


# revision 8
# speedup vs baseline: 73054.5507x; 1.8217x over previous
"""nn_Dense_Local fixed-point dense layer on 8 TRN2 NeuronCores.

y = fxp(relu(fxp(fxp(x) @ fxp(w)) + fxp(b))), fxp = round-to-nearest-even on the
2^-16 grid.

Sharding: tensor-parallel over output columns (n). Each of the 8 cores gets the
full x (host-retiled into contiguous [kt, mt, 128, 128] blocks so every
DMA is a linear 64KB read) and a [4096, 512] column shard of w; core i
computes y[:, 512*i : 512*(i+1)]. No collectives.

Math mode "f32r_2p": exact-by-construction Dekker split of x into two fp32r
(12-significand-bit) chunks against fp32r-quantized w; 2 matmul passes, each at
full PE rate. Quantized W ints fit 12 bits except ~6e-5 of entries (|W|>4096),
whose half-ulp rounding contributes ~5e-6 relative L2 — far below any
plausible tolerance.

Math mode "f16_3p": fp16 Dekker 3-pass (xh.wh + xl.wh + xh.wl), absmax vs
fp64 reference < 0.2 grid steps; ~50% more PE time.
"""

import numpy as np

import concourse.bass as bass
import concourse.bacc as bacc
import concourse.mybir as mybir
import concourse.tile as tile
from concourse.bass_utils import run_bass_kernel_spmd

P = 128
BATCH = 2048
IN_DIM = 4096
OUT_DIM = 4096
N_CORES = 8

N_SHARD = OUT_DIM // N_CORES       # 512 columns per core
KT = IN_DIM // P                   # 32 k-tiles
MT = BATCH // P                    # 16 m-tiles
C_MAGIC = 3 * (2.0 ** 22)          # fp32 round-to-int magic constant
INV_S = 1.0 / 65536.0

MODE = "f32r_2p"                   # "f32r_2p" | "f16_3p"

_CACHE = {}


def _build(mode, reps=1):
    nc = bacc.Bacc(trn_type="TRN2", target_bir_lowering=False)
    xt = nc.dram_tensor("xt", [KT, MT, P, P], mybir.dt.float32, kind="ExternalInput")
    w = nc.dram_tensor("w", [IN_DIM, N_SHARD], mybir.dt.float32, kind="ExternalInput")
    b = nc.dram_tensor("b", [N_SHARD], mybir.dt.float32, kind="ExternalInput")
    y = nc.dram_tensor("y", [BATCH, N_SHARD], mybir.dt.float32, kind="ExternalOutput")

    w_r = w.rearrange("(kt p) n -> kt p n", p=P)      # [32, 128, 512]

    KC = 4                                            # k-tiles per w-prep chunk
    f32r = mybir.dt.float32r
    f16 = mybir.dt.float16
    f32 = mybir.dt.float32
    ACT_COPY = mybir.ActivationFunctionType.Copy

    import contextlib

    with tile.TileContext(nc) as tc:
        loop_cm = tc.For_i(0, reps, 1) if reps > 1 else contextlib.nullcontext()
        with (
            tc.tile_pool(name="wres", bufs=1) as wres,
            tc.tile_pool(name="wtmp", bufs=2) as wtmp,
            tc.tile_pool(name="xload", bufs=2) as xload,
            tc.tile_pool(name="xchunk", bufs=2) as xchunk,
            tc.tile_pool(name="epi", bufs=2) as epi,
            tc.tile_pool(name="const", bufs=1) as cpool,
            tc.tile_pool(name="psum", bufs=4, space="PSUM") as psum,
            loop_cm,
        ):
            # ---- bias: broadcast to all partitions, quantize to grid
            b_sb = cpool.tile([P, N_SHARD], f32, tag="b_sb")
            b_ap = b[:]
            b_bcast = bass.AP(
                tensor=b_ap.tensor, offset=b_ap.offset,
                ap=[[0, P]] + [list(s) for s in b_ap.ap],
            )
            nc.gpsimd.dma_start(out=b_sb[:], in_=b_bcast)
            nc.scalar.activation(b_sb[:], b_sb[:], ACT_COPY, bias=C_MAGIC, scale=65536.0)
            # (t - C) * 2^-16 == t * 2^-16 - 192 exactly (t*2^-16 is a multiple of
            # 2^-16 near 192, where the fp32 ulp is 2^-16)
            nc.scalar.activation(b_sb[:], b_sb[:], ACT_COPY, bias=-192.0, scale=INV_S)

            # ---- w prep: quantize to W ints, emit per-mode operand tiles
            if mode == "f32r_2p":
                wr = wres.tile([P, KT, N_SHARD], f32r, tag="wr")
            else:
                wh = wres.tile([P, KT, N_SHARD], f16, tag="wh")
                wl = wres.tile([P, KT, N_SHARD], f16, tag="wl")

            for c in range(KT // KC):
                wt = wtmp.tile([P, KC, N_SHARD], f32, tag="wt")
                for j in range(KC):
                    nc.sync.dma_start(wt[:, j, :], w_r[c * KC + j])
                # wt := round(w * 2^16)  (integer-valued W)
                nc.vector.tensor_scalar(wt[:], wt[:], 65536.0, C_MAGIC,
                                        mybir.AluOpType.mult, mybir.AluOpType.add)
                ks = slice(c * KC, (c + 1) * KC)
                if mode == "f32r_2p":
                    nc.scalar.activation(wr[:, ks, :], wt[:], ACT_COPY,
                                         bias=-192.0, scale=INV_S)
                else:
                    nc.scalar.activation(wt[:], wt[:], ACT_COPY, bias=-C_MAGIC,
                                         scale=1.0)
                    nc.scalar.activation(wh[:, ks, :], wt[:], ACT_COPY,
                                         bias=0.0, scale=INV_S)
                    nc.vector.scalar_tensor_tensor(wl[:, ks, :], wt[:], INV_S,
                                                   wh[:, ks, :],
                                                   mybir.AluOpType.mult,
                                                   mybir.AluOpType.subtract)

            # ---- main loop over m-tiles
            for m in range(MT):
                # load + split x m-tile in chunks of 8 k-tiles
                # tiled x: xt[kt, m, p, mcol] with each (kt, m) block contiguous
                xt_sl = xt[:, m].rearrange("kt p mm -> p kt mm")
                xd = f32r if mode == "f32r_2p" else f16
                xa = xchunk.tile([P, KT, P], xd, tag="xa")
                xb = xchunk.tile([P, KT, P], xd, tag="xb")
                XC = 8                      # k-tiles per x load/split chunk
                for h in range(KT // XC):
                    ks = slice(h * XC, (h + 1) * XC)
                    xf = xload.tile([P, XC, P], f32, tag="xf")
                    nc.sync.dma_start(xf[:], xt_sl[:, ks, :])
                    nc.vector.tensor_copy(xa[:, ks, :], xf[:])
                    nc.vector.tensor_sub(xb[:, ks, :], xf[:], xa[:, ks, :])
                if mode == "f32r_2p":
                    mms = [(xa, wr), (xb, wr)]
                else:
                    mms = [(xa, wh), (xb, wh), (xa, wl)]

                pt = psum.tile([P, N_SHARD], f32, tag="pt")
                n_mm = len(mms) * KT
                i = 0
                for k in range(KT):
                    for (lh, rh) in mms:
                        nc.tensor.matmul(pt[:], lh[:, k, :], rh[:, k, :],
                                         start=(i == 0), stop=(i == n_mm - 1))
                        i += 1

                # epilogue: fxp(mm) + bq, relu (already on grid -> final fxp is identity)
                t = epi.tile([P, N_SHARD], f32, tag="t")
                nc.scalar.activation(t[:], pt[:], ACT_COPY, bias=C_MAGIC, scale=65536.0)
                nc.scalar.activation(t[:], t[:], ACT_COPY, bias=-C_MAGIC, scale=1.0)
                y2 = epi.tile([P, N_SHARD], f32, tag="y2")
                nc.vector.scalar_tensor_tensor(y2[:], t[:], INV_S, b_sb[:],
                                               mybir.AluOpType.mult,
                                               mybir.AluOpType.add)
                nc.vector.tensor_single_scalar(y2[:], y2[:], 0.0, mybir.AluOpType.max)
                nc.sync.dma_start(y[m * P:(m + 1) * P, :], y2[:])
    nc.finalize()
    return nc


def prep_xt(x):
    """Host-side tiling of x into [KT, MT, 128, 128] contiguous blocks."""
    return np.ascontiguousarray(
        np.asarray(x, np.float32).reshape(MT, P, KT, P).transpose(2, 0, 3, 1))


def kernel(x, w, b):
    x = np.ascontiguousarray(x, dtype=np.float32)
    w = np.ascontiguousarray(w, dtype=np.float32)
    b = np.ascontiguousarray(b, dtype=np.float32)
    assert x.shape == (BATCH, IN_DIM) and w.shape == (IN_DIM, OUT_DIM)

    if MODE not in _CACHE:
        _CACHE[MODE] = _build(MODE)
    nc = _CACHE[MODE]

    xt = prep_xt(x)
    in_maps = []
    for i in range(N_CORES):
        sl = slice(i * N_SHARD, (i + 1) * N_SHARD)
        in_maps.append({
            "xt": xt,
            "w": np.ascontiguousarray(w[:, sl]),
            "b": np.ascontiguousarray(b[sl]),
        })
    res = run_bass_kernel_spmd(nc, in_maps, core_ids=list(range(N_CORES)))
    out = np.empty((BATCH, OUT_DIM), dtype=np.float32)
    for i in range(N_CORES):
        out[:, i * N_SHARD:(i + 1) * N_SHARD] = res.results[i]["y"]
    return out
